# revision 23
# baseline (speedup 1.0000x reference)
"""Trainium2 Bass kernel for nn_MDLoss (retrieval_knn).

reference:
    distance[b, g, p] = ||ini_pred[b, p] - gt[b, g]||^2
    index_gt = argmin_g distance          -> [B, Np]
    gt_matched = gt[b, index_gt]          -> [B, Np, 2]
    loss = |pred - gt_matched|.mean()

Strategy (pure data-parallel over B across 8 cores, 32 instances each):
  - scores s[p, g] = 2*px*gx + 2*py*gy - (gx^2+gy^2); argmax_g s == argmin_g
    dist.  Computed on the PE as a k=11 matmul of bf16 hi/lo-split operands
    (~1e-6 absolute accuracy).  All hi/lo splits are precomputed on the host
    and DMA'd straight into the matmul operand layouts (no device prep).
  - argmax per query in 3 cheap stages instead of two full DVE passes:
      * Pool (gpsimd) folds the [128,1024] PSUM score tile twice with
        elementwise max -> h2 [128,256] in SBUF (Pool is otherwise idle).
      * DVE max8 + max_index on h2 give the folded position j (first-match
        semantics) at ~1/4 the element count.
      * the true argmin is one of {j, j+256, j+512, j+768}; all 4 candidate
        gt points are gathered per query with one multi-offset SWDGE
        indirect DMA per instance, and the winner is resolved at the end by
        recomputing the 4 exact f32 distances (bit-identical to the
        reference's (ini-gt)^2 sum) and picking the min with lowest-index
        tie-break (matching argmin's first-occurrence rule).
  - |pred - gt*| via one DVE sub + one ACT Abs with accumulate; partition
    reduce via a ones-matmul; per-core sums combined on host in float64.

Layout: 512 queries/instance as 4 tiles of 128 partitions; position t*128+p
of the P rows holds query q = 4p+t, so per-tile fold indices land in column t
of a [128, 4]-shaped tile matching the contiguous pred/ini layouts.
"""
import sys
import numpy as np

sys.path.insert(0, "/opt/trn_rl_repo")

import ml_dtypes  # noqa: E402
import concourse.bass as bass  # noqa: E402
import concourse.bacc as bacc  # noqa: E402
import concourse.tile as tile  # noqa: E402
from concourse import mybir  # noqa: E402
from concourse import bass_utils  # noqa: E402

B, NP_, NG, D = 256, 512, 1024, 2
NCORES = 8
NI = B // NCORES          # 32 instances per core
NT = NP_ // 128           # 4 query tiles per instance
NC = 8                    # fold candidates per query

f32 = mybir.dt.float32
bf16 = mybir.dt.bfloat16
u32 = mybir.dt.uint32
i32 = mybir.dt.int32
A = mybir.AluOpType
Abs = mybir.ActivationFunctionType.Abs


def _build(nc):
    PLd = nc.dram_tensor("PLd", [11, NI, NP_], bf16, kind="ExternalInput")
    GRd = nc.dram_tensor("GRd", [11, NI, NG], bf16, kind="ExternalInput")
    # GTTd[b*128 + j] = concat of the NC fold candidates gt[b, j + 128*c]
    GTTd = nc.dram_tensor("GTTd", [NI * 128, NC * 2], f32,
                          kind="ExternalInput")
    PRd = nc.dram_tensor("PRd", [NI, 128, NT * 2], f32, kind="ExternalInput")
    INd = nc.dram_tensor("INd", [NI, 128, NT * 2], f32, kind="ExternalInput")
    LOSSd = nc.dram_tensor("LOSSd", [1, 1], f32, kind="ExternalOutput")

    with tile.TileContext(nc) as tc:
        with (
            tc.tile_pool(name="sb", bufs=1) as sb,
            tc.tile_pool(name="sc", bufs=6) as sc,
            tc.tile_pool(name="ps", bufs=3, space="PSUM") as ps,
            tc.tile_pool(name="ps1", bufs=1, space="PSUM") as ps1,
        ):
            # ---------- operand loads (all host-prepped) ----------
            # chunked so instance 0's operands land first
            CHUNKS = [(0, 2), (2, 8), (8, NI)]
            Plhs = sb.tile([11, NI, NP_], bf16)
            Grhs = sb.tile([11, NI, NG], bf16)
            for lo, hi in CHUNKS:
                nc.sync.dma_start(Plhs[:, lo:hi, :], PLd[:, lo:hi, :])
                nc.sync.dma_start(Grhs[:, lo:hi, :], GRd[:, lo:hi, :])

            pred_all = sb.tile([128, NI, NT * 2], f32)
            nc.sync.dma_start(pred_all[:], PRd[:].rearrange("b p j -> p b j"))
            ini_all = sb.tile([128, NI, NT, 2], f32)
            nc.sync.dma_start(
                ini_all[:].rearrange("p b t c -> p b (t c)"),
                INd[:].rearrange("b p j -> p b j"))

            gtm = sb.tile([128, NI, NT, NC, 2], f32)
            gtsel = sb.tile([128, NI, NT, 2], f32)

            # ---------- main loop ----------
            for b in range(NI):
                jall = sc.tile([128, NT, 8], u32, tag="jall")
                for t in range(NT):
                    s = ps.tile([128, NG], f32, tag="s")
                    for h in range(2):
                        nc.tensor.matmul(
                            s[:, h * 512:(h + 1) * 512],
                            Plhs[0:11, b, t * 128:(t + 1) * 128],
                            Grhs[0:11, b, h * 512:(h + 1) * 512],
                            start=True, stop=True,
                        )
                    # second half PSUM->SBUF on ACT (DVE may read only one
                    # PSUM operand per instruction)
                    sBc = sc.tile([128, 512], f32, tag="sBc")
                    nc.scalar.copy(sBc[:], s[:, 512:1024])
                    h1 = sc.tile([128, 512], f32, tag="h1")
                    nc.vector.tensor_tensor(
                        out=h1[:], in0=s[:, 0:512], in1=sBc[:], op=A.max)
                    h3 = sc.tile([128, 128], f32, tag="h3")
                    nc.vector.tensor_reduce(
                        out=h3[:],
                        in_=h1[:].rearrange("p (c j) -> p j c", c=4),
                        axis=mybir.AxisListType.X, op=A.max)
                    top8 = sc.tile([128, 8], f32, tag="top8")
                    nc.vector.max(out=top8[:], in_=h3[:])
                    nc.vector.max_index(
                        out=jall[:, t, :], in_max=top8[:], in_values=h3[:])
                # row offsets into GTTd: j + 128*b   [128, NT] i32
                # (HW SWDGE honors one offset per partition per call; each
                # row of GTTd holds all NC candidates)
                offs = sc.tile([128, NT], i32, tag="offs")
                nc.vector.tensor_scalar(
                    out=offs[:], in0=jall[:, :, 0], scalar1=float(128 * b),
                    scalar2=None, op0=A.add)
                for t in range(NT):
                    nc.gpsimd.indirect_dma_start(
                        out=gtm[:, b, t].rearrange("p c x -> p (c x)"),
                        out_offset=None,
                        in_=GTTd[:],
                        in_offset=bass.IndirectOffsetOnAxis(
                            ap=offs[:, t:t + 1], axis=0),
                    )

                # resolve the NC fold candidates exactly, in instance
                # chunks so the work rides the pipeline instead of the tail
                if (b + 1) % 8 == 0:
                    lo, hi = b - 7, b + 1
                    w = hi - lo
                    g8 = gtm[:, lo:hi]
                    i8 = ini_all[:, lo:hi]
                    dd = sc.tile([128, 8, NT, NC, 2], f32, tag="dd")
                    nc.vector.tensor_tensor(
                        out=dd[:], in0=g8,
                        in1=i8.unsqueeze(3).broadcast_to(
                            [128, w, NT, NC, 2]),
                        op=A.subtract)
                    sq = sc.tile([128, 8, NT, NC, 2], f32, tag="sq")
                    nc.scalar.activation(
                        out=sq[:], in_=dd[:],
                        func=mybir.ActivationFunctionType.Square)
                    dsq = sc.tile([128, 8, NT, NC], f32, tag="dsq")
                    nc.vector.tensor_tensor(
                        out=dsq[:], in0=sq[:, :, :, :, 0],
                        in1=sq[:, :, :, :, 1], op=A.add)
                    mm = sc.tile([128, 8, NT], f32, tag="mmin")
                    nc.vector.tensor_reduce(
                        out=mm[:], in_=dsq[:], axis=mybir.AxisListType.X,
                        op=A.min)
                    nc.vector.tensor_copy(gtsel[:, lo:hi],
                                          g8[:, :, :, NC - 1, :])
                    for c in range(NC - 2, -1, -1):
                        mask = sc.tile([128, 8, NT], i32, tag="mask")
                        nc.vector.tensor_tensor(
                            out=mask[:], in0=dsq[:, :, :, c], in1=mm[:],
                            op=A.is_le)
                        nc.vector.copy_predicated(
                            out=gtsel[:, lo:hi],
                            mask=mask[:].unsqueeze(3).broadcast_to(
                                [128, w, NT, 2]),
                            data=g8[:, :, :, c, :])

            # ---------- final L1 loss ----------
            diff = sb.tile([128, NI * NT * 2], f32)
            nc.vector.tensor_sub(
                diff[:], pred_all[:].rearrange("p b j -> p (b j)"),
                gtsel[:].rearrange("p b t c -> p (b t c)"))
            col = sb.tile([128, 1], f32)
            nc.scalar.activation(out=diff[:], in_=diff[:], func=Abs,
                                 accum_out=col[:])
            ones = sb.tile([128, 1], f32)
            nc.vector.memset(ones[:], 1.0)
            tot_ps = ps1.tile([1, 1], f32, tag="tot")
            nc.tensor.matmul(tot_ps[:], col[:], ones[:], start=True, stop=True)
            tot_sb = sb.tile([1, 1], f32)
            nc.scalar.copy(tot_sb[:], tot_ps[:])
            nc.sync.dma_start(LOSSd[:], tot_sb[:])
    return nc


_CACHED_NC = None


def _get_nc():
    global _CACHED_NC
    if _CACHED_NC is None:
        nc = bacc.Bacc("TRN2", target_bir_lowering=False, debug=False,
                       num_devices=NCORES)
        _build(nc)
        nc.finalize()
        _CACHED_NC = nc
    return _CACHED_NC


_QPERM = np.empty(NP_, dtype=np.int64)
for _t in range(NT):
    _QPERM[_t * 128:(_t + 1) * 128] = 4 * np.arange(128) + _t


def _make_in_maps(ini_pred_poly, pred_polys_, gt_polys):
    bf = ml_dtypes.bfloat16
    ini = np.ascontiguousarray(np.asarray(ini_pred_poly, dtype=np.float32))
    pred = np.ascontiguousarray(np.asarray(pred_polys_, dtype=np.float32))
    gt = np.ascontiguousarray(np.asarray(gt_polys, dtype=np.float32))

    # ---- hi/lo splits (same math the baseline did on device) ----
    # P side: per query q: px, py in [0,1)
    P = ini[:, _QPERM, :]                      # [B, Np, 2] permuted
    Ph = P.astype(bf).astype(np.float32)
    Pl = (P - Ph).astype(bf).astype(np.float32)
    # G side rows: Gsp = [2gx, 2gy, -(gx^2+gy^2)]
    Gsp = np.empty((B, NG, 3), np.float32)
    Gsp[:, :, 0] = 2.0 * gt[:, :, 0]
    Gsp[:, :, 1] = 2.0 * gt[:, :, 1]
    gsq = np.square(gt.astype(np.float32) * np.float32(np.sqrt(0.5)))
    Gsp[:, :, 2] = -2.0 * (gsq[:, :, 0] + gsq[:, :, 1])
    Gh = Gsp.astype(bf).astype(np.float32)
    T1 = Gsp - Gh
    Gl = T1.astype(bf).astype(np.float32)
    R2l = (T1[:, :, 2] - Gl[:, :, 2]).astype(bf)
    Gh_b, Gl_b = Gh.astype(bf), Gl.astype(bf)
    Ph_b, Pl_b = Ph.astype(bf), Pl.astype(bf)
    ones_b = np.ones((B, NP_), dtype=bf)

    # lhsT rows x rhs rows (contraction pairing):
    PL = np.stack([Ph_b[:, :, 0], Ph_b[:, :, 0], Pl_b[:, :, 0], Pl_b[:, :, 0],
                   Ph_b[:, :, 1], Ph_b[:, :, 1], Pl_b[:, :, 1], Pl_b[:, :, 1],
                   ones_b, ones_b, ones_b], axis=0)          # [11, B, Np]
    GR = np.stack([Gh_b[:, :, 0], Gl_b[:, :, 0], Gh_b[:, :, 0], Gl_b[:, :, 0],
                   Gh_b[:, :, 1], Gl_b[:, :, 1], Gh_b[:, :, 1], Gl_b[:, :, 1],
                   Gh_b[:, :, 2], Gl_b[:, :, 2], R2l], axis=0)  # [11, B, Ng]

    PRf = pred[:, _QPERM, :].reshape(B, NT, 128, D).transpose(0, 2, 1, 3)
    PRf = np.ascontiguousarray(PRf.reshape(B, 128, NT * D))
    INf = ini[:, _QPERM, :].reshape(B, NT, 128, D).transpose(0, 2, 1, 3)
    INf = np.ascontiguousarray(INf.reshape(B, 128, NT * D))

    # candidate table: GTT[b, j, c, :] = gt[b, j + 128*c]
    GTT = np.ascontiguousarray(
        gt.reshape(B, NC, 128, D).transpose(0, 2, 1, 3).reshape(B * 128,
                                                                NC * D))

    in_maps = []
    for c in range(NCORES):
        sl = slice(c * NI, (c + 1) * NI)
        in_maps.append({
            "PLd": np.ascontiguousarray(PL[:, sl, :]),
            "GRd": np.ascontiguousarray(GR[:, sl, :]),
            "GTTd": np.ascontiguousarray(GTT[c * NI * 128:(c + 1) * NI * 128]),
            "PRd": np.ascontiguousarray(PRf[sl]),
            "INd": np.ascontiguousarray(INf[sl]),
        })
    return in_maps


def _run(in_maps, trace=False):
    nc = _get_nc()
    return bass_utils.run_bass_kernel_spmd(
        nc, in_maps, core_ids=list(range(NCORES)), trace=trace)


def kernel(ini_pred_poly, pred_polys_, gt_polys):
    in_maps = _make_in_maps(ini_pred_poly, pred_polys_, gt_polys)
    res = _run(in_maps)
    total = 0.0
    for c in range(NCORES):
        total += float(res.results[c]["LOSSd"][0, 0])
    return np.float32(total / (B * NP_ * D))


# revision 24
# speedup vs baseline: 1.1855x; 1.1855x over previous
"""Trainium2 Bass kernel for nn_MDLoss (retrieval_knn).

reference:
    distance[b, g, p] = ||ini_pred[b, p] - gt[b, g]||^2
    index_gt = argmin_g distance          -> [B, Np]
    gt_matched = gt[b, index_gt]          -> [B, Np, 2]
    loss = |pred - gt_matched|.mean()

Strategy (pure data-parallel over B across 8 cores, 32 instances each):
  - scores s[p, g] = 2*px*gx + 2*py*gy - (gx^2+gy^2); argmax_g s == argmin_g
    dist.  Computed on the PE as a k=11 matmul of bf16 hi/lo-split operands
    (~1e-6 absolute accuracy).  All hi/lo splits are precomputed on the host
    and DMA'd straight into the matmul operand layouts (no device prep).
  - argmax per query in 3 cheap stages instead of two full DVE passes:
      * Pool (gpsimd) folds the [128,1024] PSUM score tile twice with
        elementwise max -> h2 [128,256] in SBUF (Pool is otherwise idle).
      * DVE max8 + max_index on h2 give the folded position j (first-match
        semantics) at ~1/4 the element count.
      * the true argmin is one of {j, j+256, j+512, j+768}; all 4 candidate
        gt points are gathered per query with one multi-offset SWDGE
        indirect DMA per instance, and the winner is resolved at the end by
        recomputing the 4 exact f32 distances (bit-identical to the
        reference's (ini-gt)^2 sum) and picking the min with lowest-index
        tie-break (matching argmin's first-occurrence rule).
  - |pred - gt*| via one DVE sub + one ACT Abs with accumulate; partition
    reduce via a ones-matmul; per-core sums combined on host in float64.

Layout: 512 queries/instance as 4 tiles of 128 partitions; position t*128+p
of the P rows holds query q = 4p+t, so per-tile fold indices land in column t
of a [128, 4]-shaped tile matching the contiguous pred/ini layouts.
"""
import sys
import numpy as np

sys.path.insert(0, "/opt/trn_rl_repo")

import ml_dtypes  # noqa: E402
import concourse.bass as bass  # noqa: E402
import concourse.bacc as bacc  # noqa: E402
import concourse.tile as tile  # noqa: E402
from concourse import mybir  # noqa: E402
from concourse import bass_utils  # noqa: E402

B, NP_, NG, D = 256, 512, 1024, 2
NCORES = 8
NI = B // NCORES          # 32 instances per core
NT = NP_ // 128           # 4 query tiles per instance
NC = 8                    # fold candidates per query

f32 = mybir.dt.float32
bf16 = mybir.dt.bfloat16
u32 = mybir.dt.uint32
i32 = mybir.dt.int32
A = mybir.AluOpType
Abs = mybir.ActivationFunctionType.Abs


def _build(nc):
    PLd = nc.dram_tensor("PLd", [11, NI, NP_], bf16, kind="ExternalInput")
    GRd = nc.dram_tensor("GRd", [11, NI, NG], bf16, kind="ExternalInput")
    # GTTd[b*128 + j] = concat of the NC fold candidates gt[b, j + 128*c]
    GTTd = nc.dram_tensor("GTTd", [NI * 128, NC * 2], f32,
                          kind="ExternalInput")
    PRd = nc.dram_tensor("PRd", [NI, 128, NT * 2], f32, kind="ExternalInput")
    INd = nc.dram_tensor("INd", [NI, 128, NT * 2], f32, kind="ExternalInput")
    LOSSd = nc.dram_tensor("LOSSd", [1, 1], f32, kind="ExternalOutput")

    with tile.TileContext(nc) as tc:
        with (
            tc.tile_pool(name="sb", bufs=1) as sb,
            tc.tile_pool(name="sc", bufs=6) as sc,
            tc.tile_pool(name="ps", bufs=3, space="PSUM") as ps,
            tc.tile_pool(name="ps1", bufs=1, space="PSUM") as ps1,
        ):
            # ---------- operand loads (all host-prepped) ----------
            # chunked so instance 0's operands land first
            CHUNKS = [(0, 2), (2, 8), (8, NI)]
            Plhs = sb.tile([11, NI, NP_], bf16)
            Grhs = sb.tile([11, NI, NG], bf16)
            for lo, hi in CHUNKS:
                nc.sync.dma_start(Plhs[:, lo:hi, :], PLd[:, lo:hi, :])
                nc.sync.dma_start(Grhs[:, lo:hi, :], GRd[:, lo:hi, :])

            pred_all = sb.tile([128, NI, NT * 2], f32)
            nc.sync.dma_start(pred_all[:], PRd[:].rearrange("b p j -> p b j"))
            ini_all = sb.tile([128, NI, NT, 2], f32)
            nc.sync.dma_start(
                ini_all[:].rearrange("p b t c -> p b (t c)"),
                INd[:].rearrange("b p j -> p b j"))

            gtm = sb.tile([128, NI, NT, NC, 2], f32)
            gtsel = sb.tile([128, NI, NT, 2], f32)

            # ---------- main loop ----------
            for b in range(NI):
                jall = sc.tile([128, NT, 8], u32, tag="jall")
                for t in range(NT):
                    s = ps.tile([128, NG], f32, tag="s")
                    for h in range(2):
                        nc.tensor.matmul(
                            s[:, h * 512:(h + 1) * 512],
                            Plhs[0:11, b, t * 128:(t + 1) * 128],
                            Grhs[0:11, b, h * 512:(h + 1) * 512],
                            start=True, stop=True,
                        )
                    # second half PSUM->SBUF on ACT (DVE may read only one
                    # PSUM operand per instruction)
                    sBc = sc.tile([128, 512], f32, tag="sBc")
                    nc.scalar.copy(sBc[:], s[:, 512:1024])
                    h1 = sc.tile([128, 512], f32, tag="h1")
                    nc.vector.tensor_tensor(
                        out=h1[:], in0=s[:, 0:512], in1=sBc[:], op=A.max)
                    h2 = sc.tile([128, 256], f32, tag="h2")
                    nc.vector.tensor_tensor(
                        out=h2[:], in0=h1[:, 0:256], in1=h1[:, 256:512],
                        op=A.max)
                    h3 = sc.tile([128, 128], f32, tag="h3")
                    nc.vector.tensor_tensor(
                        out=h3[:], in0=h2[:, 0:128], in1=h2[:, 128:256],
                        op=A.max)
                    top8 = sc.tile([128, 8], f32, tag="top8")
                    nc.vector.max(out=top8[:], in_=h3[:])
                    nc.vector.max_index(
                        out=jall[:, t, :], in_max=top8[:], in_values=h3[:])
                # row offsets into GTTd: j + 128*b   [128, NT] i32
                # (HW SWDGE honors one offset per partition per call; each
                # row of GTTd holds all NC candidates)
                offs = sc.tile([128, NT], i32, tag="offs")
                nc.vector.tensor_scalar(
                    out=offs[:], in0=jall[:, :, 0], scalar1=float(128 * b),
                    scalar2=None, op0=A.add)
                for t in range(NT):
                    nc.gpsimd.indirect_dma_start(
                        out=gtm[:, b, t].rearrange("p c x -> p (c x)"),
                        out_offset=None,
                        in_=GTTd[:],
                        in_offset=bass.IndirectOffsetOnAxis(
                            ap=offs[:, t:t + 1], axis=0),
                    )

                # resolve the NC fold candidates exactly, in instance
                # chunks so the work rides the pipeline instead of the tail
                if (b + 1) % 8 == 0:
                    lo, hi = b - 7, b + 1
                    w = hi - lo
                    g8 = gtm[:, lo:hi]
                    i8 = ini_all[:, lo:hi]
                    dd = sc.tile([128, 8, NT, NC, 2], f32, tag="dd")
                    nc.vector.tensor_tensor(
                        out=dd[:], in0=g8,
                        in1=i8.unsqueeze(3).broadcast_to(
                            [128, w, NT, NC, 2]),
                        op=A.subtract)
                    sq = sc.tile([128, 8, NT, NC, 2], f32, tag="sq")
                    nc.scalar.activation(
                        out=sq[:], in_=dd[:],
                        func=mybir.ActivationFunctionType.Square)
                    dsq = sc.tile([128, 8, NT, NC], f32, tag="dsq")
                    nc.vector.tensor_tensor(
                        out=dsq[:], in0=sq[:, :, :, :, 0],
                        in1=sq[:, :, :, :, 1], op=A.add)
                    mm = sc.tile([128, 8, NT], f32, tag="mmin")
                    nc.vector.tensor_reduce(
                        out=mm[:], in_=dsq[:], axis=mybir.AxisListType.X,
                        op=A.min)
                    nc.vector.tensor_copy(gtsel[:, lo:hi],
                                          g8[:, :, :, NC - 1, :])
                    for c in range(NC - 2, -1, -1):
                        mask = sc.tile([128, 8, NT], i32, tag="mask")
                        nc.vector.tensor_tensor(
                            out=mask[:], in0=dsq[:, :, :, c], in1=mm[:],
                            op=A.is_le)
                        nc.vector.copy_predicated(
                            out=gtsel[:, lo:hi],
                            mask=mask[:].unsqueeze(3).broadcast_to(
                                [128, w, NT, 2]),
                            data=g8[:, :, :, c, :])

            # ---------- final L1 loss ----------
            diff = sb.tile([128, NI * NT * 2], f32)
            nc.vector.tensor_sub(
                diff[:], pred_all[:].rearrange("p b j -> p (b j)"),
                gtsel[:].rearrange("p b t c -> p (b t c)"))
            col = sb.tile([128, 1], f32)
            nc.scalar.activation(out=diff[:], in_=diff[:], func=Abs,
                                 accum_out=col[:])
            ones = sb.tile([128, 1], f32)
            nc.vector.memset(ones[:], 1.0)
            tot_ps = ps1.tile([1, 1], f32, tag="tot")
            nc.tensor.matmul(tot_ps[:], col[:], ones[:], start=True, stop=True)
            tot_sb = sb.tile([1, 1], f32)
            nc.scalar.copy(tot_sb[:], tot_ps[:])
            nc.sync.dma_start(LOSSd[:], tot_sb[:])
    return nc


_CACHED_NC = None


def _get_nc():
    global _CACHED_NC
    if _CACHED_NC is None:
        nc = bacc.Bacc("TRN2", target_bir_lowering=False, debug=False,
                       num_devices=NCORES)
        _build(nc)
        nc.finalize()
        _CACHED_NC = nc
    return _CACHED_NC


_QPERM = np.empty(NP_, dtype=np.int64)
for _t in range(NT):
    _QPERM[_t * 128:(_t + 1) * 128] = 4 * np.arange(128) + _t


def _make_in_maps(ini_pred_poly, pred_polys_, gt_polys):
    bf = ml_dtypes.bfloat16
    ini = np.ascontiguousarray(np.asarray(ini_pred_poly, dtype=np.float32))
    pred = np.ascontiguousarray(np.asarray(pred_polys_, dtype=np.float32))
    gt = np.ascontiguousarray(np.asarray(gt_polys, dtype=np.float32))

    # ---- hi/lo splits (same math the baseline did on device) ----
    # P side: per query q: px, py in [0,1)
    P = ini[:, _QPERM, :]                      # [B, Np, 2] permuted
    Ph = P.astype(bf).astype(np.float32)
    Pl = (P - Ph).astype(bf).astype(np.float32)
    # G side rows: Gsp = [2gx, 2gy, -(gx^2+gy^2)]
    Gsp = np.empty((B, NG, 3), np.float32)
    Gsp[:, :, 0] = 2.0 * gt[:, :, 0]
    Gsp[:, :, 1] = 2.0 * gt[:, :, 1]
    gsq = np.square(gt.astype(np.float32) * np.float32(np.sqrt(0.5)))
    Gsp[:, :, 2] = -2.0 * (gsq[:, :, 0] + gsq[:, :, 1])
    Gh = Gsp.astype(bf).astype(np.float32)
    T1 = Gsp - Gh
    Gl = T1.astype(bf).astype(np.float32)
    R2l = (T1[:, :, 2] - Gl[:, :, 2]).astype(bf)
    Gh_b, Gl_b = Gh.astype(bf), Gl.astype(bf)
    Ph_b, Pl_b = Ph.astype(bf), Pl.astype(bf)
    ones_b = np.ones((B, NP_), dtype=bf)

    # lhsT rows x rhs rows (contraction pairing):
    PL = np.stack([Ph_b[:, :, 0], Ph_b[:, :, 0], Pl_b[:, :, 0], Pl_b[:, :, 0],
                   Ph_b[:, :, 1], Ph_b[:, :, 1], Pl_b[:, :, 1], Pl_b[:, :, 1],
                   ones_b, ones_b, ones_b], axis=0)          # [11, B, Np]
    GR = np.stack([Gh_b[:, :, 0], Gl_b[:, :, 0], Gh_b[:, :, 0], Gl_b[:, :, 0],
                   Gh_b[:, :, 1], Gl_b[:, :, 1], Gh_b[:, :, 1], Gl_b[:, :, 1],
                   Gh_b[:, :, 2], Gl_b[:, :, 2], R2l], axis=0)  # [11, B, Ng]

    PRf = pred[:, _QPERM, :].reshape(B, NT, 128, D).transpose(0, 2, 1, 3)
    PRf = np.ascontiguousarray(PRf.reshape(B, 128, NT * D))
    INf = ini[:, _QPERM, :].reshape(B, NT, 128, D).transpose(0, 2, 1, 3)
    INf = np.ascontiguousarray(INf.reshape(B, 128, NT * D))

    # candidate table: GTT[b, j, c, :] = gt[b, j + 128*c]
    GTT = np.ascontiguousarray(
        gt.reshape(B, NC, 128, D).transpose(0, 2, 1, 3).reshape(B * 128,
                                                                NC * D))

    in_maps = []
    for c in range(NCORES):
        sl = slice(c * NI, (c + 1) * NI)
        in_maps.append({
            "PLd": np.ascontiguousarray(PL[:, sl, :]),
            "GRd": np.ascontiguousarray(GR[:, sl, :]),
            "GTTd": np.ascontiguousarray(GTT[c * NI * 128:(c + 1) * NI * 128]),
            "PRd": np.ascontiguousarray(PRf[sl]),
            "INd": np.ascontiguousarray(INf[sl]),
        })
    return in_maps


def _run(in_maps, trace=False):
    nc = _get_nc()
    return bass_utils.run_bass_kernel_spmd(
        nc, in_maps, core_ids=list(range(NCORES)), trace=trace)


def kernel(ini_pred_poly, pred_polys_, gt_polys):
    in_maps = _make_in_maps(ini_pred_poly, pred_polys_, gt_polys)
    res = _run(in_maps)
    total = 0.0
    for c in range(NCORES):
        total += float(res.results[c]["LOSSd"][0, 0])
    return np.float32(total / (B * NP_ * D))
